# revision 1
# baseline (speedup 1.0000x reference)
"""Trainium2 8-core kernel for 2-layer GAT (nn_DiGCN_65335042507185).

Design (v2):
  Nodes are partitioned across 8 cores by dst (12500/core). Per core, dst
  nodes are bin-packed by in-degree into 392 groups of 32 nodes; each group
  owns 5 edge-tiles of 128 slots (640 capacity). Four NEFFs per call:

    A  (lin, F=128): xs1 = x@W1 and attention preacts s1,d1 on device.
    B  (agg, relu):  layer-1 edge softmax + one-hot aggregation.
    B2 (lin, F=64):  xs2 = h@W2 and preacts s2,d2 on device.
    C  (agg):        layer-2 aggregation -> final embeddings.

  The agg NEFF streams host-gathered xs[src] rows (64 feats + ones col,
  bf16) plus per-slot preact/dstloc. On device: LeakyReLU+exp (softmax
  numerator), a 32-wide one-hot built at DVE 2x mode ([P,G,TC] layout with
  materialized iota), weight folded into the one-hot, 32-col TensorE
  matmuls with tile_position packing 4 groups per PSUM bank, and
  normalization Relu(z^-1 * agg) on ScalarE. Host does graph partitioning,
  slot layout, gathers (halo exchange surrogate), and resharding only.
"""
import sys
for _p in ("/opt/trn_rl_repo", "/root/.axon_site/_ro/trn_rl_repo"):
    if _p not in sys.path:
        sys.path.insert(0, _p)

import numpy as np
import ml_dtypes
from contextlib import ExitStack

import concourse.bass as bass
import concourse.bacc as bacc
import concourse.mybir as mybir
import concourse.tile as tile
from concourse.bass_utils import run_bass_kernel_spmd

P = 128
N = 100_000
NFEAT = 128
NHID = 64
NEG_SLOPE = 0.2
NCORES = 8
NSH = 12500                  # real nodes per core
G = 32                       # dst nodes per group (one-hot width)
TPG = 5                      # tiles per group (640 edge slots capacity)
NGRP = 392                   # groups per core
NODES_PAD = NGRP * G         # 12544 node slots per core
NT = NGRP * TPG              # 1960 tiles per core
NSLOT = NT * P               # 250880 edge slots per core
GPC = 56                     # groups per chunk
TC = GPC * TPG               # 140 tiles per chunk
NCHUNK = NGRP // GPC         # 14
NPS = NGRP // 4              # 98 psum tiles (4 groups each)
PPC = GPC // 4               # 7 psum tiles per chunk
CS = 65                      # stream cols: 64 feats + ones
AF = mybir.ActivationFunctionType
DT = mybir.dt
ALU = mybir.AluOpType
BF16 = ml_dtypes.bfloat16
FP8 = False
F8 = ml_dtypes.float8_e4m3

_CACHE = {}


# ---------------------------------------------------------------- device ----

def _build_lin(F):
    """xs = x@W plus preacts s,d. In: xT [F,NODES_PAD] bf16, W [F,64] bf16,
    WT [64,F] bf16, apair [64,2] bf16. Out: xs_sd [66, NODES_PAD] bf16
    (rows 0:64 = xs^T, 64:66 = s,d)."""
    nc = bacc.Bacc("TRN2", target_bir_lowering=False, debug=False,
                   num_devices=NCORES)
    xT = nc.dram_tensor("xT", [F, NODES_PAD], DT.bfloat16,
                        kind="ExternalInput").ap()
    w_h = nc.dram_tensor("w", [F, NHID], DT.bfloat16, kind="ExternalInput").ap()
    wT_h = nc.dram_tensor("wT", [NHID, F], DT.bfloat16, kind="ExternalInput").ap()
    ap_h = nc.dram_tensor("apair", [NHID, 2], DT.bfloat16, kind="ExternalInput").ap()
    out_h = nc.dram_tensor("xs_sd", [NHID + 2, NODES_PAD], DT.bfloat16,
                           kind="ExternalOutput").ap()
    NTILE = NODES_PAD // P          # 98
    CHT = 14                        # node tiles per input DMA chunk
    with tile.TileContext(nc) as tc, ExitStack() as ctx:
        cpool = ctx.enter_context(tc.tile_pool(name="consts", bufs=1))
        wcat = cpool.tile([F, NHID + 2], DT.bfloat16)
        nc.sync.dma_start(wcat[:, 0:NHID], w_h[:])
        wT = cpool.tile([NHID, F], DT.bfloat16)
        nc.sync.dma_start(wT[:], wT_h[:])
        apair = cpool.tile([NHID, 2], DT.bfloat16)
        nc.sync.dma_start(apair[:], ap_h[:])
        with tc.tile_pool(name="va", bufs=1, space="PSUM") as vpool:
            va_ps = vpool.tile([F, 2], DT.float32)
            nc.tensor.matmul(va_ps[:], lhsT=wT[:], rhs=apair[:],
                             start=True, stop=True)
            nc.vector.tensor_copy(wcat[:, NHID:NHID + 2], va_ps[:])

        xp = ctx.enter_context(tc.tile_pool(name="x", bufs=3))
        stp = ctx.enter_context(tc.tile_pool(name="stage", bufs=3))
        pp = ctx.enter_context(tc.tile_pool(name="ps", bufs=8, space="PSUM"))
        MMW = 2 * P                       # rhs cols per matmul
        for ci in range(NTILE // CHT):
            xt = xp.tile([F, CHT * P], DT.bfloat16, tag="xt")
            nc.sync.dma_start(xt[:], xT[:, ci * CHT * P:(ci + 1) * CHT * P])
            stage = stp.tile([NHID + 2, CHT * P], DT.bfloat16, tag="stage")
            for k in range(CHT * P // MMW):
                c0 = k * MMW
                ps = pp.tile([NHID + 2, MMW], DT.float32, tag="ps")
                nc.tensor.matmul(ps[:], lhsT=wcat[:],
                                 rhs=xt[:, k * MMW:(k + 1) * MMW],
                                 start=True, stop=True)
                if k % 2 == 0:
                    nc.vector.tensor_copy(stage[:, c0:c0 + MMW], ps[:])
                else:
                    nc.scalar.activation(stage[:, c0:c0 + MMW], ps[:], AF.Copy)
            nc.sync.dma_start(out_h[:, ci * CHT * P:(ci + 1) * CHT * P], stage[:])
    nc.compile()
    return nc


def _build_agg(relu, f32_out):
    """One GAT aggregation layer over the packed edge stream."""
    nc = bacc.Bacc("TRN2", target_bir_lowering=False, debug=False,
                   num_devices=NCORES)
    sdt = DT.float8e4 if FP8 else DT.bfloat16
    feats = nc.dram_tensor("feats", [P, NT, CS], sdt,
                           kind="ExternalInput").ap()
    pre_h = nc.dram_tensor("pre", [P, NT], DT.bfloat16, kind="ExternalInput").ap()
    dst_h = nc.dram_tensor("dstloc", [P, NT], DT.bfloat16,
                           kind="ExternalInput").ap()
    iota_h = nc.dram_tensor("iota", [P, G, TC], DT.bfloat16,
                            kind="ExternalInput").ap()
    odt = DT.float32 if f32_out else DT.bfloat16
    out_h = nc.dram_tensor("out", [P, NPS, NHID], odt, kind="ExternalOutput").ap()

    with tile.TileContext(nc) as tc, ExitStack() as ctx:
        cpool = ctx.enter_context(tc.tile_pool(name="consts", bufs=1))
        iota = cpool.tile([P, G, TC], DT.bfloat16)
        nc.sync.dma_start(iota[:], iota_h[:])

        sp = ctx.enter_context(tc.tile_pool(name="stream", bufs=2))
        mp = ctx.enter_context(tc.tile_pool(name="meta", bufs=2))
        ep = ctx.enter_context(tc.tile_pool(name="edge", bufs=2))
        mwp = ctx.enter_context(tc.tile_pool(name="mw", bufs=2))
        op = ctx.enter_context(tc.tile_pool(name="out", bufs=2))
        zp = ctx.enter_context(tc.tile_pool(name="z", bufs=8))
        pp = ctx.enter_context(tc.tile_pool(name="ps", bufs=8, space="PSUM"))

        def _evac(p):
            pl, pci = p
            outsb = op.tile([P, PPC, NHID], odt, tag="outsb")
            for k, ps in enumerate(pl):
                zinv = zp.tile([P, 1], DT.float32, tag="zinv")
                nc.vector.reciprocal(zinv[:], ps[:, NHID:NHID + 1])
                nc.scalar.activation(outsb[:, k, :], ps[:, 0:NHID],
                                     AF.Relu if relu else AF.Copy,
                                     scale=zinv[:])
            nc.sync.dma_start(out_h[:, pci * PPC:(pci + 1) * PPC, :], outsb[:])

        pend = None
        for ci in range(NCHUNK):
            t0 = ci * TC
            S = sp.tile([P, TC, CS], sdt, tag="S")
            nc.sync.dma_start(S[:], feats[:, t0:t0 + TC, :])
            pre = mp.tile([P, TC], DT.bfloat16, tag="pre")
            nc.sync.dma_start(pre[:], pre_h[:, t0:t0 + TC])
            dstl = mp.tile([P, TC], DT.bfloat16, tag="dstl")
            nc.sync.dma_start(dstl[:], dst_h[:, t0:t0 + TC])

            lk = ep.tile([P, TC], DT.float32, tag="lk")
            nc.vector.tensor_scalar(out=lk[:], in0=pre[:], scalar1=NEG_SLOPE,
                                    scalar2=None, op0=ALU.mult)
            nc.vector.tensor_tensor(out=lk[:], in0=lk[:], in1=pre[:], op=ALU.max)
            w = ep.tile([P, TC], DT.bfloat16, tag="w")
            nc.scalar.activation(w[:], lk[:], AF.Exp)

            M = mwp.tile([P, G, TC], DT.bfloat16, tag="M")
            nc.vector.tensor_tensor(
                out=M[:], in0=dstl[:, None, :].broadcast_to([P, G, TC]),
                in1=iota[:], op=ALU.is_equal)
            Mw = mwp.tile([P, G, TC], DT.bfloat16, tag="Mw")
            nc.vector.tensor_tensor(
                out=Mw[:], in0=M[:], in1=w[:, None, :].broadcast_to([P, G, TC]),
                op=ALU.mult)

            if pend is not None:
                _evac(pend)
            ps_list = []
            for k in range(PPC):
                ps = pp.tile([P, CS], DT.float32, tag="ps")
                for j in range(4):
                    gl = k * 4 + j
                    tt = gl * TPG
                    for t in range(TPG):
                        nc.tensor.matmul(ps[G * j:G * (j + 1), :],
                                         lhsT=Mw[:, :, tt + t],
                                         rhs=S[:, tt + t, :],
                                         start=(t == 0), stop=(t == TPG - 1),
                                         tile_position=(0, G * j))
                ps_list.append(ps)
            pend = (ps_list, ci)
        _evac(pend)
    nc.compile()
    return nc


def _get(key, builder, *a):
    if key not in _CACHE:
        _CACHE[key] = builder(*a)
    return _CACHE[key]


# ------------------------------------------------------------------ host ----

def _bin_pack(deg):
    """LPT: assign NSH nodes to NGRP bins of exactly G slots, load<=TPG*P.
    Returns perm [NGRP*G] int32 (node id or -1 for pad)."""
    import heapq
    order = np.argsort(-deg, kind="stable")
    heap = [(0, g) for g in range(NGRP)]
    heapq.heapify(heap)
    bins = [[] for _ in range(NGRP)]
    spill = []
    for n in order:
        d = int(deg[n])
        load, g = heapq.heappop(heap)
        while len(bins[g]) >= G:
            load, g = heapq.heappop(heap)
        bins[g].append(n)
        nl = load + d
        if nl > TPG * P:
            raise RuntimeError(f"bin overflow {nl}")
        if len(bins[g]) < G:
            heapq.heappush(heap, (nl, g))
        else:
            spill.append((nl, g))
    perm = np.full(NGRP * G, -1, dtype=np.int64)
    for g, lst in enumerate(bins):
        perm[g * G:g * G + len(lst)] = lst
    return perm


def _prep_graph(edge_index):
    """Per-core slot layout. Returns list of dicts."""
    ei = np.asarray(edge_index)
    src = np.concatenate([ei[0], np.arange(N, dtype=ei.dtype)]).astype(np.int64)
    dst = np.concatenate([ei[1], np.arange(N, dtype=ei.dtype)]).astype(np.int64)
    owner = dst // NSH
    cores = []
    for c in range(NCORES):
        sel = owner == c
        s_c = src[sel]
        d_c = dst[sel] - c * NSH                     # local dst 0..12499
        deg = np.bincount(d_c, minlength=NSH)
        perm = _bin_pack(deg)                        # [12544] node or -1
        # node -> (group, j)
        slot_of_node = np.full(NSH, -1, dtype=np.int64)
        valid = perm >= 0
        slot_of_node[perm[valid]] = np.nonzero(valid)[0]
        key = slot_of_node[d_c]                      # g*32+j per edge
        order = np.argsort(key, kind="stable")
        s_c, d_c, key = s_c[order], d_c[order], key[order]
        grp = key // G
        # position within group: running index
        gstart = np.searchsorted(grp, np.arange(NGRP))
        cnt = np.diff(np.append(gstart, len(grp)))
        if cnt.max() > TPG * P:
            raise RuntimeError(f"group overflow {cnt.max()}")
        pos = np.arange(len(grp)) - gstart[grp]
        slot = grp * (TPG * P) + pos                 # linear slot in [0, NSLOT)
        slot_src = np.zeros(NSLOT, dtype=np.int64)
        slot_dst_g = np.zeros(NSLOT, dtype=np.int64) # global dst per slot
        dstloc = np.zeros(NSLOT, dtype=np.float32)
        wkill = np.full(NSLOT, True)                 # pad slots
        slot_src[slot] = s_c
        slot_dst_g[slot] = d_c + c * NSH
        dstloc[slot] = key % G
        wkill[slot] = False
        cores.append(dict(slot_src=slot_src, slot_dst=slot_dst_g,
                          dstloc=dstloc.astype(BF16), wkill=wkill, perm=perm))
    return cores


def _make_iota():
    i = np.arange(G, dtype=np.float32)[None, :, None]
    return np.broadcast_to(i, (P, G, TC)).astype(BF16).copy()


def _feats_stream(table66, core):
    """table66 [N,65] (col 64 = 1.0). -> [P, NT, CS] stream dtype."""
    flat = table66[core["slot_src"]]                 # [NSLOT, 65]
    flat[core["wkill"], 64] = 0                      # ones col 0 on pad slots
    return np.ascontiguousarray(
        flat.reshape(NT, P, CS).transpose(1, 0, 2))


def _meta_streams(pre_f32, core):
    pre = pre_f32.copy()
    pre[core["wkill"]] = -30000.0
    pre = pre.astype(BF16).reshape(NT, P).T.copy()
    dstl = core["dstloc"].reshape(NT, P).T.copy()
    return pre, dstl


def _run_lin(nc_lin, xT_list, W, a_src, a_dst):
    Wb = np.ascontiguousarray(W, dtype=np.float32).astype(BF16)
    WTb = np.ascontiguousarray(W.T, dtype=np.float32).astype(BF16)
    ap = np.stack([a_src, a_dst], axis=1).astype(np.float32).astype(BF16)
    in_maps = [{"xT": xT_list[c], "w": Wb, "wT": WTb, "apair": ap}
               for c in range(NCORES)]
    res = run_bass_kernel_spmd(nc_lin, in_maps, core_ids=list(range(NCORES)))
    # assemble global tables: xs [N,64] bf16 (from cols 0:NSH), s,d [N] f32
    xs = np.empty((N, NHID + 2), dtype=np.float32)
    for c in range(NCORES):
        xs[c * NSH:(c + 1) * NSH] = \
            res.results[c]["xs_sd"][:, :NSH].T.astype(np.float32)
    return xs[:, 0:NHID], xs[:, NHID], xs[:, NHID + 1]


def _run_agg(nc_agg, cores, xs, s, d, iota):
    table66 = np.empty((N, CS), dtype=np.float32)
    table66[:, 0:NHID] = xs
    table66[:, NHID] = 1.0
    table66 = table66.astype(F8 if FP8 else BF16)
    in_maps = []
    for core in cores:
        pre = s[core["slot_src"]] + d[core["slot_dst"]]
        pre_st, dst_st = _meta_streams(pre, core)
        in_maps.append({"feats": _feats_stream(table66, core),
                        "pre": pre_st, "dstloc": dst_st, "iota": iota})
    res = run_bass_kernel_spmd(nc_agg, in_maps, core_ids=list(range(NCORES)))
    # out [P, NPS, 64] -> rows r = pstile*128+p = g*32+j -> node perm[g*32+j]
    full = np.empty((N, NHID), dtype=np.float32)
    for c, core in enumerate(cores):
        o = res.results[c]["out"]                   # [P, NPS, 64]
        rows = o.transpose(1, 0, 2).reshape(NODES_PAD, NHID).astype(np.float32)
        valid = core["perm"] >= 0
        full[c * NSH + core["perm"][valid]] = rows[valid]
    return full


def kernel(x, W1, att_src1, att_dst1, W2, att_src2, att_dst2, edge_index):
    x = np.asarray(x, dtype=np.float32)
    W1 = np.asarray(W1, dtype=np.float32)
    W2 = np.asarray(W2, dtype=np.float32)
    a_s1 = np.asarray(att_src1, dtype=np.float32)
    a_d1 = np.asarray(att_dst1, dtype=np.float32)
    a_s2 = np.asarray(att_src2, dtype=np.float32)
    a_d2 = np.asarray(att_dst2, dtype=np.float32)

    cores = _prep_graph(edge_index)
    iota = _make_iota()

    ncA = _get(("lin", NFEAT), _build_lin, NFEAT)
    ncB2 = _get(("lin", NHID), _build_lin, NHID)
    ncB = _get(("agg", True), _build_agg, True, False)
    ncC = _get(("agg", False), _build_agg, False, True)

    # layer 1
    xb = x.astype(BF16)
    xT_list = []
    for c in range(NCORES):
        xt = np.zeros((NFEAT, NODES_PAD), dtype=BF16)
        xt[:, :NSH] = xb[c * NSH:(c + 1) * NSH].T
        xT_list.append(xt)
    xs1, s1, d1 = _run_lin(ncA, xT_list, W1, a_s1, a_d1)
    h = _run_agg(ncB, cores, xs1, s1, d1, iota)

    # layer 2
    hb = h.astype(BF16)
    hT_list = []
    for c in range(NCORES):
        ht = np.zeros((NHID, NODES_PAD), dtype=BF16)
        ht[:, :NSH] = hb[c * NSH:(c + 1) * NSH].T
        hT_list.append(ht)
    xs2, s2, d2 = _run_lin(ncB2, hT_list, W2, a_s2, a_d2)
    out = _run_agg(ncC, cores, xs2, s2, d2, iota)
    return out.astype(np.float32)



# revision 8
# speedup vs baseline: 1.6419x; 1.6419x over previous
"""Trainium2 8-core kernel for 2-layer GAT (nn_DiGCN_65335042507185).

Design (v3):
  Nodes partitioned across 8 cores by dst (12500/core). Per core, dst nodes
  are packed into 392 variable-capacity bins (<=32 nodes each); bin g owns
  caps[g] whole 128-slot edge tiles, with bin loads packed close to capacity
  (count-aware exact-fill greedy), giving ~1692 tiles/core vs 1960 for the
  uniform layout. The schedule (caps, chunking) is common to all 8 cores so
  one SPMD program serves all of them.

  Four NEFFs per call:
    A  (lin, F=128): xs1 = x@W1 + attention preacts s1,d1.
    B  (agg, relu):  layer-1 edge softmax + one-hot aggregation.
    B2 (lin, F=64):  xs2 = h@W2 + preacts s2,d2.
    C  (agg):        layer-2 aggregation -> final embeddings.

  The agg NEFF streams host-gathered xs[src] rows quantized to fp8 e3m4 with
  per-row power-of-two scales (exact in fp): col 64 carries the scale 2^k so
  the z (softmax denominator) accumulates exactly; 2^-k is folded into the
  edge weight w on device via a bf16 yinv stream. The one-hot weight matrix
  is built two ways, split across engines to balance load: gpsimd
  local_scatter (w scattered by int16 combined indices) for ~half the
  chunks, DVE is_equal+mult against an iota for the rest. 32-col TensorE
  matmuls with tile_position pack 4 bins per PSUM tile; evacuation keeps z
  in f32 (Act relu-copy, one DVE reciprocal + broadcast-mult per chunk).
  Host does graph partitioning, slot layout, gathers (halo exchange
  surrogate), quantization, and resharding only.
"""
import sys
for _p in ("/opt/trn_rl_repo", "/root/.axon_site/_ro/trn_rl_repo"):
    if _p not in sys.path:
        sys.path.insert(0, _p)

import bisect
import numpy as np
import ml_dtypes
from contextlib import ExitStack

import concourse.bass as bass
import concourse.bacc as bacc
import concourse.mybir as mybir
import concourse.tile as tile
from concourse.bass_utils import run_bass_kernel_spmd

P = 128
N = 100_000
NFEAT = 128
NHID = 64
NEG_SLOPE = 0.2
NCORES = 8
NSH = 12500                  # real nodes per core
G = 32                       # dst nodes per bin (one-hot width)
NB = 392                     # bins per core (multiple of 4)
NPS = NB // 4                # psum tiles (4 bins each)
CS = 65                      # stream cols: 64 feats + scale col
SLACK_T = 25                 # extra tiles over the per-core ceil floor
MINCAP = 3                   # min tiles per bin (tail feasibility)
TC_T = 176                   # target tiles per chunk
SUBT = 62                    # tiles per local_scatter call (62*32=1984<2046)
POOL_FRAC = 0.50             # fraction of tiles handled by gpsimd scatter
FP8_L1 = True                # layer-1 agg feature stream in fp8 e3m4
FP8_L2 = True                # layer-2 agg feature stream in fp8 e3m4

AF = mybir.ActivationFunctionType
DT = mybir.dt
ALU = mybir.AluOpType
BF16 = ml_dtypes.bfloat16
F8E3 = ml_dtypes.float8_e3m4

_CACHE = {}


# ------------------------------------------------------------- scheduling ----

def _make_caps(degs):
    """Common per-bin tile capacities from the cross-core degree-rank
    profile. caps sorted desc by construction."""
    prof = np.zeros(NB)
    for dg in degs:
        sd = np.sort(dg)[::-1]
        prof += np.pad(sd, (0, NB * G - NSH)).reshape(NB, G).sum(1)
    prof /= len(degs)
    capsf = prof / 128.0
    caps = np.maximum(np.round(capsf), MINCAP).astype(int)
    NT_need = max(int(np.ceil(d.sum() / 128)) for d in degs) + SLACK_T
    resid = capsf - caps
    while caps.sum() < NT_need:
        i = int(np.argmax(resid)); caps[i] += 1; resid[i] -= 1
    while caps.sum() > NT_need:
        cand = np.where(caps > MINCAP)[0]
        i = cand[np.argmin(resid[cand])]
        caps[i] -= 1; resid[i] += 1
    return caps


def _pack_core(deg, caps):
    """Count-aware exact-fill greedy. Returns perm [NB*G] (node or -1)."""
    order = np.argsort(deg, kind="stable")
    pool_deg = deg[order].astype(np.int64).tolist()
    pool_node = order.tolist()
    nbins = len(caps)
    nodes_left = len(pool_node)
    perm = np.full(NB * G, -1, dtype=np.int64)
    for bi, cap in enumerate(caps):
        cnt = min(G, int(np.ceil(nodes_left / (nbins - bi))))
        target = int(cap) * 128
        load = 0
        members = []
        for k in range(cnt):
            if not pool_deg:
                break
            r = cnt - k
            ideal = (target - load) / r
            i = bisect.bisect_right(pool_deg, ideal) - 1
            if i < 0:
                i = 0
            if r == 1:
                j = bisect.bisect_right(pool_deg, target - load) - 1
                if j >= 0:
                    i = j
            load += pool_deg.pop(i)
            members.append(pool_node.pop(i))
        if load > target:
            raise RuntimeError(f"bin {bi} overfull {load}>{target}")
        nodes_left -= len(members)
        perm[bi * G:bi * G + len(members)] = members
    if pool_node:
        raise RuntimeError(f"{len(pool_node)} nodes unplaced")
    return perm


def _make_schedule(degs):
    """Common schedule: caps + chunk list. Chunks are contiguous psum-tile
    ranges; each chunk is handled by the gpsimd scatter path ('pool') or the
    DVE is_equal path ('dve')."""
    caps = _make_caps(degs)
    NT = int(caps.sum())
    # tile offset of each bin
    bin_t0 = np.concatenate([[0], np.cumsum(caps)])
    # psum tile -> tile span
    ps_t0 = [int(bin_t0[4 * q]) for q in range(NPS)] + [NT]
    # chunks: greedy accumulate psum tiles up to TC_T
    spans = []
    q = 0
    while q < NPS:
        q0 = q
        while q < NPS and ps_t0[q + 1] - ps_t0[q0] <= TC_T:
            q += 1
        spans.append((q0, q))
    # assign kinds balancing DVE vs Pool busy (ns/tile: dve 34.3, pool 44.4)
    chunks = []
    dve_ns = pool_ns = 0.0
    idx_off = 0
    dstl_off = 0
    for (q0, q1) in spans:
        t0, t1 = ps_t0[q0], ps_t0[q1]
        TC = t1 - t0
        take_pool = (pool_ns + TC * 44.4) * (1 - POOL_FRAC) <= \
                    (dve_ns + TC * 34.3) * POOL_FRAC
        ch = dict(q0=q0, q1=q1, t0=t0, TC=TC)
        # per-bin local tile lists
        bins = []
        for b in range(4 * q0, 4 * q1):
            lo = int(bin_t0[b]) - t0
            bins.append((b % 4, lo, int(caps[b])))
        ch["bins"] = bins
        if take_pool:
            pool_ns += TC * 44.4
            ch["kind"] = "pool"
            subs = []
            tl = 0
            c0 = idx_off
            while tl < TC:
                sT = min(SUBT, TC - tl)
                icols = sT + (sT % 2)
                subs.append((tl, sT, c0 - idx_off, icols))
                c0 += icols
                tl += sT
            ch["subs"] = subs
            ch["idx_off"] = idx_off
            ch["icols"] = c0 - idx_off
            idx_off = c0
        else:
            dve_ns += TC * 34.3
            ch["kind"] = "dve"
            ch["dstl_off"] = dstl_off
            dstl_off += TC
        chunks.append(ch)
    return dict(caps=caps, NT=NT, chunks=chunks, NIDX=max(idx_off, 2),
                NDVE=max(dstl_off, 2),
                TCMAX=max(c["TC"] for c in chunks),
                TCMAXD=max([c["TC"] for c in chunks if c["kind"] == "dve"],
                           default=2),
                PPCMAX=max(c["q1"] - c["q0"] for c in chunks))


# ---------------------------------------------------------------- device ----

def _build_lin(F):
    """xs = x@W plus preacts s,d. In: xT [F,NB*G] bf16, W [F,64] bf16,
    WT [64,F] bf16, apair [64,2] bf16. Out: xs_sd [66, NB*G] bf16."""
    NODES_PAD = NB * G
    nc = bacc.Bacc("TRN2", target_bir_lowering=False, debug=False,
                   num_devices=NCORES)
    xT = nc.dram_tensor("xT", [F, NODES_PAD], DT.bfloat16,
                        kind="ExternalInput").ap()
    w_h = nc.dram_tensor("w", [F, NHID], DT.bfloat16, kind="ExternalInput").ap()
    wT_h = nc.dram_tensor("wT", [NHID, F], DT.bfloat16, kind="ExternalInput").ap()
    ap_h = nc.dram_tensor("apair", [NHID, 2], DT.bfloat16, kind="ExternalInput").ap()
    out_h = nc.dram_tensor("xs_sd", [NHID + 2, NODES_PAD], DT.bfloat16,
                           kind="ExternalOutput").ap()
    NTILE = NODES_PAD // P          # 98
    CHT = 14                        # node tiles per input DMA chunk
    with tile.TileContext(nc) as tc, ExitStack() as ctx:
        cpool = ctx.enter_context(tc.tile_pool(name="consts", bufs=1))
        wcat = cpool.tile([F, NHID + 2], DT.bfloat16)
        nc.sync.dma_start(wcat[:, 0:NHID], w_h[:])
        wT = cpool.tile([NHID, F], DT.bfloat16)
        nc.sync.dma_start(wT[:], wT_h[:])
        apair = cpool.tile([NHID, 2], DT.bfloat16)
        nc.sync.dma_start(apair[:], ap_h[:])
        with tc.tile_pool(name="va", bufs=1, space="PSUM") as vpool:
            va_ps = vpool.tile([F, 2], DT.float32)
            nc.tensor.matmul(va_ps[:], lhsT=wT[:], rhs=apair[:],
                             start=True, stop=True)
            nc.vector.tensor_copy(wcat[:, NHID:NHID + 2], va_ps[:])

        xp = ctx.enter_context(tc.tile_pool(name="x", bufs=3))
        stp = ctx.enter_context(tc.tile_pool(name="stage", bufs=3))
        pp = ctx.enter_context(tc.tile_pool(name="ps", bufs=8, space="PSUM"))
        MMW = 2 * P                       # rhs cols per matmul
        for ci in range(NTILE // CHT):
            xt = xp.tile([F, CHT * P], DT.bfloat16, tag="xt")
            nc.scalar.dma_start(xt[:], xT[:, ci * CHT * P:(ci + 1) * CHT * P])
            stage = stp.tile([NHID + 2, CHT * P], DT.bfloat16, tag="stage")
            for k in range(CHT * P // MMW):
                c0 = k * MMW
                ps = pp.tile([NHID + 2, MMW], DT.float32, tag="ps")
                nc.tensor.matmul(ps[:], lhsT=wcat[:],
                                 rhs=xt[:, k * MMW:(k + 1) * MMW],
                                 start=True, stop=True)
                if k % 2 == 0:
                    nc.vector.tensor_copy(stage[:, c0:c0 + MMW], ps[:])
                else:
                    nc.scalar.activation(stage[:, c0:c0 + MMW], ps[:], AF.Copy)
            nc.sync.dma_start(out_h[:, ci * CHT * P:(ci + 1) * CHT * P],
                              stage[:])
    nc.compile()
    return nc


def _build_agg(relu, fp8, sched):
    """One GAT aggregation layer over the packed edge stream."""
    NT = sched["NT"]
    TCMAX, TCMAXD, PPCMAX = sched["TCMAX"], sched["TCMAXD"], sched["PPCMAX"]
    nc = bacc.Bacc("TRN2", target_bir_lowering=False, debug=False,
                   num_devices=NCORES)
    sdt = DT.float8e3 if fp8 else DT.bfloat16
    feats = nc.dram_tensor("feats", [P, NT, CS], sdt,
                           kind="ExternalInput").ap()
    meta_h = nc.dram_tensor("meta", [P, 2 * NT], DT.bfloat16,
                            kind="ExternalInput").ap()
    idx_h = nc.dram_tensor("idx", [P, sched["NIDX"]], DT.int16,
                           kind="ExternalInput").ap()
    dstl_h = nc.dram_tensor("dstl", [P, sched["NDVE"]], DT.bfloat16,
                            kind="ExternalInput").ap()
    iota_h = nc.dram_tensor("iota", [P, G, TCMAXD], DT.bfloat16,
                            kind="ExternalInput").ap()
    out_h = nc.dram_tensor("out", [P, NPS, NHID], DT.bfloat16,
                           kind="ExternalOutput").ap()
    ICMAX = max([c["icols"] for c in sched["chunks"] if c["kind"] == "pool"],
                default=2)

    with tile.TileContext(nc) as tc, ExitStack() as ctx:
        cpool = ctx.enter_context(tc.tile_pool(name="consts", bufs=1))
        iota = cpool.tile([P, G, TCMAXD], DT.bfloat16)
        nc.sync.dma_start(iota[:], iota_h[:])

        sp = ctx.enter_context(tc.tile_pool(name="stream", bufs=2))
        mp = ctx.enter_context(tc.tile_pool(name="meta", bufs=2))
        ip = ctx.enter_context(tc.tile_pool(name="idx", bufs=2))
        dp = ctx.enter_context(tc.tile_pool(name="dstl", bufs=2))
        wpool = ctx.enter_context(tc.tile_pool(name="w", bufs=2))
        mwp = ctx.enter_context(tc.tile_pool(name="mwp", bufs=2))
        mwd = ctx.enter_context(tc.tile_pool(name="mwd", bufs=2))
        op = ctx.enter_context(tc.tile_pool(name="out", bufs=2))
        onp = ctx.enter_context(tc.tile_pool(name="outn", bufs=2))
        zp = ctx.enter_context(tc.tile_pool(name="z", bufs=4))
        pp = ctx.enter_context(tc.tile_pool(name="ps", bufs=8, space="PSUM"))

        for ch in sched["chunks"]:
            t0, TC = ch["t0"], ch["TC"]
            PPC = ch["q1"] - ch["q0"]
            S = sp.tile([P, TCMAX, CS], sdt, tag="S")
            h1 = TC // 2
            nc.sync.dma_start(S[:, 0:h1, :], feats[:, t0:t0 + h1, :])
            nc.sync.dma_start(S[:, h1:TC, :], feats[:, t0 + h1:t0 + TC, :])
            meta = mp.tile([P, 2 * TCMAX], DT.bfloat16, tag="meta")
            nc.scalar.dma_start(meta[:, 0:2 * TC],
                                meta_h[:, 2 * t0:2 * t0 + 2 * TC])
            pre = meta[:, 0:TC]
            yinv = meta[:, TC:2 * TC]

            lk = wpool.tile([P, TCMAX], DT.float32, tag="lk")
            nc.vector.tensor_scalar(out=lk[:, 0:TC], in0=pre, scalar1=NEG_SLOPE,
                                    scalar2=None, op0=ALU.mult)
            nc.vector.tensor_tensor(out=lk[:, 0:TC], in0=lk[:, 0:TC], in1=pre,
                                    op=ALU.max)
            w = wpool.tile([P, TCMAX + 2], DT.bfloat16, tag="w")
            nc.scalar.activation(w[:, 0:TC], lk[:, 0:TC], AF.Exp)
            wp = wpool.tile([P, TCMAX + 2], DT.bfloat16, tag="wp")
            nc.vector.tensor_tensor(out=wp[:, 0:TC], in0=w[:, 0:TC], in1=yinv,
                                    op=ALU.mult)

            if ch["kind"] == "pool":
                idxt = ip.tile([P, ICMAX], DT.int16, tag="idxt")
                nc.sync.dma_start(idxt[:, 0:ch["icols"]],
                                  idx_h[:, ch["idx_off"]:ch["idx_off"] + ch["icols"]])
                Mw = mwp.tile([P, TCMAX * G], DT.bfloat16, tag="Mw")
                for (tl, sT, ic0, icols) in ch["subs"]:
                    nc.gpsimd.local_scatter(
                        Mw[:, tl * G:(tl + sT) * G],
                        wp[:, tl:tl + icols],
                        idxt[:, ic0:ic0 + icols],
                        channels=P, num_elems=sT * G, num_idxs=icols)

                def lhsT(tl):
                    return Mw[:, tl * G:(tl + 1) * G]
            else:
                dstlt = dp.tile([P, TCMAXD], DT.bfloat16, tag="dstlt")
                nc.sync.dma_start(dstlt[:, 0:TC],
                                  dstl_h[:, ch["dstl_off"]:ch["dstl_off"] + TC])
                M = mwd.tile([P, G, TCMAXD], DT.bfloat16, tag="M")
                nc.vector.tensor_tensor(
                    out=M[:, :, 0:TC],
                    in0=dstlt[:, None, 0:TC].broadcast_to([P, G, TC]),
                    in1=iota[:, :, 0:TC], op=ALU.is_equal)
                nc.vector.tensor_tensor(
                    out=M[:, :, 0:TC], in0=M[:, :, 0:TC],
                    in1=wp[:, None, 0:TC].broadcast_to([P, G, TC]),
                    op=ALU.mult)

                def lhsT(tl):
                    return M[:, :, tl]

            outsb = op.tile([P, PPCMAX, CS], DT.float32, tag="outsb")
            for ql in range(PPC):
                ps = pp.tile([P, CS], DT.float32, tag="ps")
                for (j4, lo, ntil) in ch["bins"][4 * ql:4 * ql + 4]:
                    for k in range(ntil):
                        nc.tensor.matmul(ps[G * j4:G * (j4 + 1), :],
                                         lhsT=lhsT(lo + k),
                                         rhs=S[:, lo + k, :],
                                         start=(k == 0), stop=(k == ntil - 1),
                                         tile_position=(0, G * j4))
                nc.scalar.activation(outsb[:, ql, :], ps[:],
                                     AF.Relu if relu else AF.Copy)
            zinv = zp.tile([P, PPCMAX, 1], DT.float32, tag="zinv")
            nc.vector.reciprocal(zinv[:, 0:PPC, :],
                                 outsb[:, 0:PPC, NHID:NHID + 1])
            outn = onp.tile([P, PPCMAX, NHID], DT.bfloat16, tag="outn")
            nc.vector.tensor_tensor(
                out=outn[:, 0:PPC, :], in0=outsb[:, 0:PPC, 0:NHID],
                in1=zinv[:, 0:PPC, :].broadcast_to([P, PPC, NHID]),
                op=ALU.mult)
            nc.scalar.dma_start(out_h[:, ch["q0"]:ch["q1"], :],
                                outn[:, 0:PPC, :])
    nc.compile()
    return nc


def _get(key, builder, *a):
    if key not in _CACHE:
        _CACHE[key] = builder(*a)
    return _CACHE[key]


# ------------------------------------------------------------------ host ----

def _prep_graph(edge_index):
    """Returns (sched, cores). Per core: slot arrays + node perm."""
    ei = np.asarray(edge_index)
    src = np.concatenate([ei[0], np.arange(N, dtype=ei.dtype)]).astype(np.int64)
    dst = np.concatenate([ei[1], np.arange(N, dtype=ei.dtype)]).astype(np.int64)
    owner = dst // NSH
    degs = []
    per_core = []
    for c in range(NCORES):
        sel = owner == c
        s_c, d_c = src[sel], dst[sel] - c * NSH
        degs.append(np.bincount(d_c, minlength=NSH))
        per_core.append((s_c, d_c))
    sched = _make_schedule(degs)
    caps = sched["caps"]
    NT = sched["NT"]
    NSLOT = NT * P
    bin_t0 = np.concatenate([[0], np.cumsum(caps)])   # tile offset per bin
    cores = []
    for c in range(NCORES):
        s_c, d_c = per_core[c]
        perm = _pack_core(degs[c], caps)              # [NB*G] node or -1
        slot_of_node = np.full(NSH, -1, dtype=np.int64)
        valid = perm >= 0
        slot_of_node[perm[valid]] = np.nonzero(valid)[0]
        key = slot_of_node[d_c]                       # bin*G + j per edge
        order = np.argsort(key, kind="stable")
        s_c, d_c, key = s_c[order], d_c[order], key[order]
        binid = key // G
        bstart = np.searchsorted(binid, np.arange(NB))
        cnt = np.diff(np.append(bstart, len(binid)))
        if (cnt > caps * 128).any():
            raise RuntimeError("bin capacity overflow")
        pos = np.arange(len(binid)) - bstart[binid]
        slot = (bin_t0[binid] * 128 + pos)            # linear slot
        slot_src = np.zeros(NSLOT, dtype=np.int64)
        slot_dst_g = np.zeros(NSLOT, dtype=np.int64)
        slot_j = np.zeros(NSLOT, dtype=np.int64)
        pad = np.full(NSLOT, True)
        slot_src[slot] = s_c
        slot_dst_g[slot] = d_c + c * NSH
        slot_j[slot] = key % G
        pad[slot] = False
        cores.append(dict(slot_src=slot_src, slot_dst=slot_dst_g,
                          slot_j=slot_j, pad=pad, perm=perm))
    return sched, cores


def _quant_table(xs, fp8):
    """xs [N,64] f32 -> (table [N,65] stream dtype, yinv [N] bf16-exact)."""
    if not fp8:
        t = np.empty((N, CS), dtype=np.float32)
        t[:, 0:NHID] = xs
        t[:, NHID] = 1.0
        return t.astype(BF16), np.ones(N, dtype=np.float32)
    mx = np.abs(xs).max(axis=1)
    k = np.where(mx > 0, 3 - np.ceil(np.log2(np.maximum(mx, 1e-30))), 0.0)
    k = np.clip(k, -3, 3)
    sc = np.exp2(k).astype(np.float32)
    t = np.empty((N, CS), dtype=np.float32)
    t[:, 0:NHID] = xs * sc[:, None]
    t[:, NHID] = sc
    return t.astype(F8E3), (1.0 / sc)


def _streams(core, sched, table, yinv_n, s_n, d_n):
    """Build feats/meta/idx/dstl arrays for one core."""
    NT = sched["NT"]
    ssrc = core["slot_src"]
    feats = table[ssrc]                                   # [NSLOT, 65]
    feats = np.ascontiguousarray(
        feats.reshape(NT, P, CS).transpose(1, 0, 2))      # [P, NT, CS]
    pre = (s_n[ssrc] + d_n[core["slot_dst"]]).astype(np.float32)
    pre[core["pad"]] = -30000.0
    pre = pre.astype(BF16).reshape(NT, P).T               # [P, NT]
    yv = yinv_n[ssrc].astype(BF16).reshape(NT, P).T       # [P, NT]
    jj = core["slot_j"].reshape(NT, P).T                  # [P, NT]
    padm = core["pad"].reshape(NT, P).T
    meta = np.empty((P, 2 * NT), dtype=BF16)
    idx = np.full((P, sched["NIDX"]), -1, dtype=np.int16)
    dstl = np.zeros((P, sched["NDVE"]), dtype=BF16)
    for ch in sched["chunks"]:
        t0, TC = ch["t0"], ch["TC"]
        meta[:, 2 * t0:2 * t0 + TC] = pre[:, t0:t0 + TC]
        meta[:, 2 * t0 + TC:2 * t0 + 2 * TC] = yv[:, t0:t0 + TC]
        if ch["kind"] == "pool":
            for (tl, sT, ic0, icols) in ch["subs"]:
                a = t0 + tl
                v = (np.arange(sT)[None, :] * G + jj[:, a:a + sT]).astype(np.int16)
                v[padm[:, a:a + sT]] = -1
                idx[:, ch["idx_off"] + ic0:ch["idx_off"] + ic0 + sT] = v
        else:
            dstl[:, ch["dstl_off"]:ch["dstl_off"] + TC] = \
                jj[:, t0:t0 + TC].astype(BF16)
    return dict(feats=feats, meta=meta, idx=idx, dstl=dstl)


def _make_iota(sched):
    i = np.arange(G, dtype=np.float32)[None, :, None]
    return np.broadcast_to(i, (P, G, sched["TCMAXD"])).astype(BF16).copy()


def _run_lin(nc_lin, xT_list, W, a_src, a_dst):
    Wb = np.ascontiguousarray(W, dtype=np.float32).astype(BF16)
    WTb = np.ascontiguousarray(W.T, dtype=np.float32).astype(BF16)
    ap = np.stack([a_src, a_dst], axis=1).astype(np.float32).astype(BF16)
    in_maps = [{"xT": xT_list[c], "w": Wb, "wT": WTb, "apair": ap}
               for c in range(NCORES)]
    res = run_bass_kernel_spmd(nc_lin, in_maps, core_ids=list(range(NCORES)))
    xs = np.empty((N, NHID + 2), dtype=np.float32)
    for c in range(NCORES):
        xs[c * NSH:(c + 1) * NSH] = \
            res.results[c]["xs_sd"][:, :NSH].T.astype(np.float32)
    return xs[:, 0:NHID], xs[:, NHID], xs[:, NHID + 1]


def _run_agg(nc_agg, sched, cores, xs, s, d, fp8, iota):
    table, yinv_n = _quant_table(xs, fp8)
    in_maps = []
    for core in cores:
        st = _streams(core, sched, table, yinv_n, s, d)
        st["iota"] = iota
        in_maps.append(st)
    res = run_bass_kernel_spmd(nc_agg, in_maps, core_ids=list(range(NCORES)))
    full = np.zeros((N, NHID), dtype=np.float32)
    for c, core in enumerate(cores):
        o = res.results[c]["out"]                     # [P, NPS, 64] bf16
        rows = o.transpose(1, 0, 2).reshape(NB * G, NHID).astype(np.float32)
        valid = core["perm"] >= 0
        full[c * NSH + core["perm"][valid]] = rows[valid]
    return full


def kernel(x, W1, att_src1, att_dst1, W2, att_src2, att_dst2, edge_index):
    x = np.asarray(x, dtype=np.float32)
    W1 = np.asarray(W1, dtype=np.float32)
    W2 = np.asarray(W2, dtype=np.float32)
    a_s1 = np.asarray(att_src1, dtype=np.float32)
    a_d1 = np.asarray(att_dst1, dtype=np.float32)
    a_s2 = np.asarray(att_src2, dtype=np.float32)
    a_d2 = np.asarray(att_dst2, dtype=np.float32)

    sched, cores = _prep_graph(edge_index)
    iota = _make_iota(sched)
    NODES_PAD = NB * G

    ncA = _get(("lin", NFEAT), _build_lin, NFEAT)
    ncB2 = _get(("lin", NHID), _build_lin, NHID)
    ncB = _get(("agg", True), _build_agg, True, FP8_L1, sched)
    ncC = _get(("agg", False), _build_agg, False, FP8_L2, sched)

    # layer 1
    xb = x.astype(BF16)
    xT_list = []
    for c in range(NCORES):
        xt = np.zeros((NFEAT, NODES_PAD), dtype=BF16)
        xt[:, :NSH] = xb[c * NSH:(c + 1) * NSH].T
        xT_list.append(xt)
    xs1, s1, d1 = _run_lin(ncA, xT_list, W1, a_s1, a_d1)
    h = _run_agg(ncB, sched, cores, xs1, s1, d1, FP8_L1, iota)

    # layer 2
    hb = h.astype(BF16)
    hT_list = []
    for c in range(NCORES):
        ht = np.zeros((NHID, NODES_PAD), dtype=BF16)
        ht[:, :NSH] = hb[c * NSH:(c + 1) * NSH].T
        hT_list.append(ht)
    xs2, s2, d2 = _run_lin(ncB2, hT_list, W2, a_s2, a_d2)
    out = _run_agg(ncC, sched, cores, xs2, s2, d2, FP8_L2, iota)
    return out.astype(np.float32)


# revision 17
# speedup vs baseline: 1.7196x; 1.0473x over previous
"""Trainium2 8-core kernel for 2-layer GAT (nn_DiGCN_65335042507185).

Design (v3):
  Nodes partitioned across 8 cores by dst (12500/core). Per core, dst nodes
  are packed into 392 variable-capacity bins (<=32 nodes each); bin g owns
  caps[g] whole 128-slot edge tiles, with bin loads packed close to capacity
  (count-aware exact-fill greedy), giving ~1692 tiles/core vs 1960 for the
  uniform layout. The schedule (caps, chunking) is common to all 8 cores so
  one SPMD program serves all of them.

  Four NEFFs per call:
    A  (lin, F=128): xs1 = x@W1 + attention preacts s1,d1.
    B  (agg, relu):  layer-1 edge softmax + one-hot aggregation.
    B2 (lin, F=64):  xs2 = h@W2 + preacts s2,d2.
    C  (agg):        layer-2 aggregation -> final embeddings.

  The agg NEFF streams host-gathered xs[src] rows quantized to fp8 e3m4 with
  per-row power-of-two scales (exact in fp): col 64 carries the scale 2^k so
  the z (softmax denominator) accumulates exactly; 2^-k is folded into the
  edge weight w on device via a bf16 yinv stream. The one-hot weight matrix
  is built two ways, split across engines to balance load: gpsimd
  local_scatter (w scattered by int16 combined indices) for ~half the
  chunks, DVE is_equal+mult against an iota for the rest. 32-col TensorE
  matmuls with tile_position pack 4 bins per PSUM tile; evacuation keeps z
  in f32 (Act relu-copy, one DVE reciprocal + broadcast-mult per chunk).
  Host does graph partitioning, slot layout, gathers (halo exchange
  surrogate), quantization, and resharding only.
"""
import sys
for _p in ("/opt/trn_rl_repo", "/root/.axon_site/_ro/trn_rl_repo"):
    if _p not in sys.path:
        sys.path.insert(0, _p)

import bisect
import numpy as np
import ml_dtypes
from contextlib import ExitStack

import concourse.bass as bass
import concourse.bacc as bacc
import concourse.mybir as mybir
import concourse.tile as tile
from concourse.bass_utils import run_bass_kernel_spmd

P = 128
N = 100_000
NFEAT = 128
NHID = 64
NEG_SLOPE = 0.2
NCORES = 8
NSH = 12500                  # real nodes per core
G = 32                       # dst nodes per bin (one-hot width)
NB = 392                     # bins per core (multiple of 4)
NPS = NB // 4                # psum tiles (4 bins each)
CS = 65                      # stream cols: 64 feats + scale col
SLACK_T = 25                 # extra tiles over the per-core ceil floor
MINCAP = 3                   # min tiles per bin (tail feasibility)
TC_T = 140                   # target tiles per chunk (~8 psum tiles)
SUBT = 62                    # tiles per local_scatter call (62*32=1984<2046)
POOL_FRAC = 0.50             # fraction of tiles handled by gpsimd scatter
FP8_L1 = True                # layer-1 agg feature stream in fp8 e3m4
FP8_L2 = True                # layer-2 agg feature stream in fp8 e3m4

AF = mybir.ActivationFunctionType
DT = mybir.dt
ALU = mybir.AluOpType
BF16 = ml_dtypes.bfloat16
F8E3 = ml_dtypes.float8_e3m4

_CACHE = {}


# ------------------------------------------------------------- scheduling ----

def _make_caps(degs):
    """Common per-bin tile capacities from the cross-core degree-rank
    profile. caps sorted desc by construction."""
    prof = np.zeros(NB)
    for dg in degs:
        sd = np.sort(dg)[::-1]
        prof += np.pad(sd, (0, NB * G - NSH)).reshape(NB, G).sum(1)
    prof /= len(degs)
    capsf = prof / 128.0
    caps = np.maximum(np.round(capsf), MINCAP).astype(int)
    NT_need = max(int(np.ceil(d.sum() / 128)) for d in degs) + SLACK_T
    resid = capsf - caps
    while caps.sum() < NT_need:
        i = int(np.argmax(resid)); caps[i] += 1; resid[i] -= 1
    while caps.sum() > NT_need:
        cand = np.where(caps > MINCAP)[0]
        i = cand[np.argmin(resid[cand])]
        caps[i] -= 1; resid[i] += 1
    return caps


def _pack_core(deg, caps):
    """Count-aware exact-fill greedy. Returns perm [NB*G] (node or -1)."""
    order = np.argsort(deg, kind="stable")
    pool_deg = deg[order].astype(np.int64).tolist()
    pool_node = order.tolist()
    nbins = len(caps)
    nodes_left = len(pool_node)
    perm = np.full(NB * G, -1, dtype=np.int64)
    for bi, cap in enumerate(caps):
        cnt = min(G, int(np.ceil(nodes_left / (nbins - bi))))
        target = int(cap) * 128
        load = 0
        members = []
        for k in range(cnt):
            if not pool_deg:
                break
            r = cnt - k
            ideal = (target - load) / r
            i = bisect.bisect_right(pool_deg, ideal) - 1
            if i < 0:
                i = 0
            if r == 1:
                j = bisect.bisect_right(pool_deg, target - load) - 1
                if j >= 0:
                    i = j
            load += pool_deg.pop(i)
            members.append(pool_node.pop(i))
        if load > target:
            raise RuntimeError(f"bin {bi} overfull {load}>{target}")
        nodes_left -= len(members)
        perm[bi * G:bi * G + len(members)] = members
    if pool_node:
        raise RuntimeError(f"{len(pool_node)} nodes unplaced")
    return perm


def _make_schedule(degs):
    """Common schedule: caps + chunk list. Chunks are contiguous psum-tile
    ranges; each chunk is handled by the gpsimd scatter path ('pool') or the
    DVE is_equal path ('dve')."""
    caps = _make_caps(degs)
    NT = int(caps.sum())
    # tile offset of each bin
    bin_t0 = np.concatenate([[0], np.cumsum(caps)])
    # psum tile -> tile span
    ps_t0 = [int(bin_t0[4 * q]) for q in range(NPS)] + [NT]
    # chunks: greedy accumulate psum tiles up to TC_T
    spans = []
    q = 0
    while q < NPS:
        q0 = q
        while q < NPS and ps_t0[q + 1] - ps_t0[q0] <= TC_T:
            q += 1
        spans.append((q0, q))
    # assign kinds balancing DVE vs Pool busy (ns/tile: dve 34.3, pool 44.4)
    # dve_ns starts with the DVE's fixed extras (w-prep, evac recip+mult) so
    # the split accounts for them AND the first chunks go to the pool path
    # (overlapping the iota load needed by the dve path).
    chunks = []
    dve_ns, pool_ns = 14000.0, 0.0
    idx_off = 0
    dstl_off = 0
    for (q0, q1) in spans:
        t0, t1 = ps_t0[q0], ps_t0[q1]
        TC = t1 - t0
        take_pool = pool_ns + TC * 44.4 <= dve_ns + TC * 34.3
        ch = dict(q0=q0, q1=q1, t0=t0, TC=TC)
        # per-bin local tile lists
        bins = []
        for b in range(4 * q0, 4 * q1):
            lo = int(bin_t0[b]) - t0
            bins.append((b % 4, lo, int(caps[b])))
        ch["bins"] = bins
        if take_pool:
            pool_ns += TC * 44.4
            ch["kind"] = "pool"
            subs = []
            tl = 0
            c0 = idx_off
            while tl < TC:
                sT = min(SUBT, TC - tl)
                icols = sT + (sT % 2)
                subs.append((tl, sT, c0 - idx_off, icols))
                c0 += icols
                tl += sT
            ch["subs"] = subs
            ch["idx_off"] = idx_off
            ch["icols"] = c0 - idx_off
            idx_off = c0
        else:
            dve_ns += TC * 34.3
            ch["kind"] = "dve"
            ch["dstl_off"] = dstl_off
            dstl_off += TC
        chunks.append(ch)
    return dict(caps=caps, NT=NT, chunks=chunks, NIDX=max(idx_off, 2),
                NDVE=max(dstl_off, 2),
                TCMAX=max(c["TC"] for c in chunks),
                TCMAXD=max([c["TC"] for c in chunks if c["kind"] == "dve"],
                           default=2),
                PPCMAX=max(c["q1"] - c["q0"] for c in chunks))


# ---------------------------------------------------------------- device ----

def _build_lin(F, fp8=False):
    """xs = x@W plus preacts s,d. In: xT [F,NB*G] fp8/bf16, W [F,64] bf16,
    WT [64,F] bf16, apair [64,2] bf16. Out: xs_sd [66, NB*G] bf16."""
    NODES_PAD = NB * G
    nc = bacc.Bacc("TRN2", target_bir_lowering=False, debug=False,
                   num_devices=NCORES)
    xdt = DT.float8e3 if fp8 else DT.bfloat16
    xT = nc.dram_tensor("xT", [F, NODES_PAD], xdt,
                        kind="ExternalInput").ap()
    w_h = nc.dram_tensor("w", [F, NHID], DT.bfloat16, kind="ExternalInput").ap()
    wT_h = nc.dram_tensor("wT", [NHID, F], DT.bfloat16, kind="ExternalInput").ap()
    ap_h = nc.dram_tensor("apair", [NHID, 2], DT.bfloat16, kind="ExternalInput").ap()
    out_h = nc.dram_tensor("xs_sd", [NHID + 2, NODES_PAD], DT.bfloat16,
                           kind="ExternalOutput").ap()
    NTILE = NODES_PAD // P          # 98
    CHT = 14                        # node tiles per input DMA chunk
    with tile.TileContext(nc) as tc, ExitStack() as ctx:
        cpool = ctx.enter_context(tc.tile_pool(name="consts", bufs=1))
        wcat = cpool.tile([F, NHID + 2], DT.bfloat16)
        nc.sync.dma_start(wcat[:, 0:NHID], w_h[:])
        wT = cpool.tile([NHID, F], DT.bfloat16)
        nc.sync.dma_start(wT[:], wT_h[:])
        apair = cpool.tile([NHID, 2], DT.bfloat16)
        nc.sync.dma_start(apair[:], ap_h[:])
        with tc.tile_pool(name="va", bufs=1, space="PSUM") as vpool:
            va_ps = vpool.tile([F, 2], DT.float32)
            nc.tensor.matmul(va_ps[:], lhsT=wT[:], rhs=apair[:],
                             start=True, stop=True)
            nc.vector.tensor_copy(wcat[:, NHID:NHID + 2], va_ps[:])

        xp = ctx.enter_context(tc.tile_pool(name="x", bufs=3))
        stp = ctx.enter_context(tc.tile_pool(name="stage", bufs=3))
        pp = ctx.enter_context(tc.tile_pool(name="ps", bufs=8, space="PSUM"))
        MMW = 2 * P                       # rhs cols per matmul
        for ci in range(NTILE // CHT):
            xt = xp.tile([F, CHT * P], xdt, tag="xt")
            nc.scalar.dma_start(xt[:], xT[:, ci * CHT * P:(ci + 1) * CHT * P])
            stage = stp.tile([NHID + 2, CHT * P], DT.bfloat16, tag="stage")
            for k in range(CHT * P // MMW):
                c0 = k * MMW
                ps = pp.tile([NHID + 2, MMW], DT.float32, tag="ps")
                nc.tensor.matmul(ps[:], lhsT=wcat[:],
                                 rhs=xt[:, k * MMW:(k + 1) * MMW],
                                 start=True, stop=True)
                if k % 2 == 0:
                    nc.vector.tensor_copy(stage[:, c0:c0 + MMW], ps[:])
                else:
                    nc.scalar.activation(stage[:, c0:c0 + MMW], ps[:], AF.Copy)
            nc.sync.dma_start(out_h[:, ci * CHT * P:(ci + 1) * CHT * P],
                              stage[:])
    nc.compile()
    return nc


def _build_agg(relu, fp8, sched):
    """One GAT aggregation layer over the packed edge stream."""
    NT = sched["NT"]
    TCMAX, TCMAXD, PPCMAX = sched["TCMAX"], sched["TCMAXD"], sched["PPCMAX"]
    nc = bacc.Bacc("TRN2", target_bir_lowering=False, debug=False,
                   num_devices=NCORES)
    sdt = DT.float8e3 if fp8 else DT.bfloat16
    feats = nc.dram_tensor("feats", [P, NT, CS], sdt,
                           kind="ExternalInput").ap()
    meta_h = nc.dram_tensor("meta", [P, 2 * NT], DT.bfloat16,
                            kind="ExternalInput").ap()
    idx_h = nc.dram_tensor("idx", [P, sched["NIDX"]], DT.int16,
                           kind="ExternalInput").ap()
    dstl_h = nc.dram_tensor("dstl", [P, sched["NDVE"]], DT.bfloat16,
                            kind="ExternalInput").ap()
    iota_h = nc.dram_tensor("iota", [P, G, TCMAXD], DT.bfloat16,
                            kind="ExternalInput").ap()
    out_h = nc.dram_tensor("out", [P, NPS, NHID], DT.bfloat16,
                           kind="ExternalOutput").ap()
    ICMAX = max([c["icols"] for c in sched["chunks"] if c["kind"] == "pool"],
                default=2)

    with tile.TileContext(nc) as tc, ExitStack() as ctx:
        cpool = ctx.enter_context(tc.tile_pool(name="consts", bufs=1))
        iota = cpool.tile([P, G, TCMAXD], DT.bfloat16)
        nc.sync.dma_start(iota[:], iota_h[:])

        sp = ctx.enter_context(tc.tile_pool(name="stream", bufs=3))
        mp = ctx.enter_context(tc.tile_pool(name="meta", bufs=3))
        ip = ctx.enter_context(tc.tile_pool(name="idx", bufs=3))
        dp = ctx.enter_context(tc.tile_pool(name="dstl", bufs=3))
        wpool = ctx.enter_context(tc.tile_pool(name="w", bufs=3))
        mwp = ctx.enter_context(tc.tile_pool(name="mwp", bufs=3))
        mwd = ctx.enter_context(tc.tile_pool(name="mwd", bufs=3))
        op = ctx.enter_context(tc.tile_pool(name="out", bufs=2))
        onp = ctx.enter_context(tc.tile_pool(name="outn", bufs=2))
        zp = ctx.enter_context(tc.tile_pool(name="z", bufs=4))
        pp = ctx.enter_context(tc.tile_pool(name="ps", bufs=8, space="PSUM"))

        for ch in sched["chunks"]:
            t0, TC = ch["t0"], ch["TC"]
            PPC = ch["q1"] - ch["q0"]
            S = sp.tile([P, TCMAX, CS], sdt, tag="S")
            nc.sync.dma_start(S[:, 0:TC, :], feats[:, t0:t0 + TC, :])
            meta = mp.tile([P, 2 * TCMAX], DT.bfloat16, tag="meta")
            nc.scalar.dma_start(meta[:, 0:2 * TC],
                                meta_h[:, 2 * t0:2 * t0 + 2 * TC])
            pre = meta[:, 0:TC]
            yinv = meta[:, TC:2 * TC]

            lk = wpool.tile([P, TCMAX], DT.float32, tag="lk")
            nc.vector.tensor_scalar(out=lk[:, 0:TC], in0=pre, scalar1=NEG_SLOPE,
                                    scalar2=None, op0=ALU.mult)
            nc.vector.tensor_tensor(out=lk[:, 0:TC], in0=lk[:, 0:TC], in1=pre,
                                    op=ALU.max)
            w = wpool.tile([P, TCMAX + 2], DT.bfloat16, tag="w")
            nc.scalar.activation(w[:, 0:TC], lk[:, 0:TC], AF.Exp)
            wp = wpool.tile([P, TCMAX + 2], DT.bfloat16, tag="wp")
            nc.vector.tensor_tensor(out=wp[:, 0:TC], in0=w[:, 0:TC], in1=yinv,
                                    op=ALU.mult)

            if ch["kind"] == "pool":
                idxt = ip.tile([P, ICMAX], DT.int16, tag="idxt")
                nc.sync.dma_start(idxt[:, 0:ch["icols"]],
                                  idx_h[:, ch["idx_off"]:ch["idx_off"] + ch["icols"]])
                Mw = mwp.tile([P, TCMAX * G], DT.bfloat16, tag="Mw")
                for (tl, sT, ic0, icols) in ch["subs"]:
                    nc.gpsimd.local_scatter(
                        Mw[:, tl * G:(tl + sT) * G],
                        wp[:, tl:tl + icols],
                        idxt[:, ic0:ic0 + icols],
                        channels=P, num_elems=sT * G, num_idxs=icols)

                def lhsT(tl):
                    return Mw[:, tl * G:(tl + 1) * G]
            else:
                dstlt = dp.tile([P, TCMAXD], DT.bfloat16, tag="dstlt")
                nc.sync.dma_start(dstlt[:, 0:TC],
                                  dstl_h[:, ch["dstl_off"]:ch["dstl_off"] + TC])
                M = mwd.tile([P, G, TCMAXD], DT.bfloat16, tag="M")
                nc.vector.tensor_tensor(
                    out=M[:, :, 0:TC],
                    in0=dstlt[:, None, 0:TC].broadcast_to([P, G, TC]),
                    in1=iota[:, :, 0:TC], op=ALU.is_equal)
                nc.vector.tensor_tensor(
                    out=M[:, :, 0:TC], in0=M[:, :, 0:TC],
                    in1=wp[:, None, 0:TC].broadcast_to([P, G, TC]),
                    op=ALU.mult)

                def lhsT(tl):
                    return M[:, :, tl]

            outsb = op.tile([P, PPCMAX, CS], DT.float32, tag="outsb")
            for ql in range(PPC):
                ps = pp.tile([P, CS], DT.float32, tag="ps")
                for (j4, lo, ntil) in ch["bins"][4 * ql:4 * ql + 4]:
                    for k in range(ntil):
                        nc.tensor.matmul(ps[G * j4:G * (j4 + 1), :],
                                         lhsT=lhsT(lo + k),
                                         rhs=S[:, lo + k, :],
                                         start=(k == 0), stop=(k == ntil - 1),
                                         tile_position=(0, G * j4))
                nc.scalar.activation(outsb[:, ql, :], ps[:],
                                     AF.Relu if relu else AF.Copy)
            zinv = zp.tile([P, PPCMAX, 1], DT.float32, tag="zinv")
            nc.vector.reciprocal(zinv[:, 0:PPC, :],
                                 outsb[:, 0:PPC, NHID:NHID + 1])
            outn = onp.tile([P, PPCMAX, NHID], DT.bfloat16, tag="outn")
            nc.vector.tensor_tensor(
                out=outn[:, 0:PPC, :], in0=outsb[:, 0:PPC, 0:NHID],
                in1=zinv[:, 0:PPC, :].broadcast_to([P, PPC, NHID]),
                op=ALU.mult)
            nc.scalar.dma_start(out_h[:, ch["q0"]:ch["q1"], :],
                                outn[:, 0:PPC, :])
    nc.compile()
    return nc


def _get(key, builder, *a):
    if key not in _CACHE:
        _CACHE[key] = builder(*a)
    return _CACHE[key]


# ------------------------------------------------------------------ host ----

def _prep_graph(edge_index):
    """Returns (sched, cores). Per core: slot arrays + node perm."""
    ei = np.asarray(edge_index)
    src = np.concatenate([ei[0], np.arange(N, dtype=ei.dtype)]).astype(np.int64)
    dst = np.concatenate([ei[1], np.arange(N, dtype=ei.dtype)]).astype(np.int64)
    owner = dst // NSH
    degs = []
    per_core = []
    for c in range(NCORES):
        sel = owner == c
        s_c, d_c = src[sel], dst[sel] - c * NSH
        degs.append(np.bincount(d_c, minlength=NSH))
        per_core.append((s_c, d_c))
    sched = _make_schedule(degs)
    caps = sched["caps"]
    NT = sched["NT"]
    NSLOT = NT * P
    bin_t0 = np.concatenate([[0], np.cumsum(caps)])   # tile offset per bin
    cores = []
    for c in range(NCORES):
        s_c, d_c = per_core[c]
        perm = _pack_core(degs[c], caps)              # [NB*G] node or -1
        slot_of_node = np.full(NSH, -1, dtype=np.int64)
        valid = perm >= 0
        slot_of_node[perm[valid]] = np.nonzero(valid)[0]
        key = slot_of_node[d_c]                       # bin*G + j per edge
        order = np.argsort(key, kind="stable")
        s_c, d_c, key = s_c[order], d_c[order], key[order]
        binid = key // G
        bstart = np.searchsorted(binid, np.arange(NB))
        cnt = np.diff(np.append(bstart, len(binid)))
        if (cnt > caps * 128).any():
            raise RuntimeError("bin capacity overflow")
        pos = np.arange(len(binid)) - bstart[binid]
        slot = (bin_t0[binid] * 128 + pos)            # linear slot
        slot_src = np.zeros(NSLOT, dtype=np.int64)
        slot_dst_g = np.zeros(NSLOT, dtype=np.int64)
        slot_j = np.zeros(NSLOT, dtype=np.int64)
        pad = np.full(NSLOT, True)
        slot_src[slot] = s_c
        slot_dst_g[slot] = d_c + c * NSH
        slot_j[slot] = key % G
        pad[slot] = False
        cores.append(dict(slot_src=slot_src, slot_dst=slot_dst_g,
                          slot_j=slot_j, pad=pad, perm=perm))
    return sched, cores


def _quant_table(xs, fp8):
    """xs [N,64] f32 -> (table [N,65] stream dtype, yinv [N] bf16-exact)."""
    if not fp8:
        t = np.empty((N, CS), dtype=np.float32)
        t[:, 0:NHID] = xs
        t[:, NHID] = 1.0
        return t.astype(BF16), np.ones(N, dtype=np.float32)
    mx = np.abs(xs).max(axis=1)
    k = np.where(mx > 0, 3 - np.ceil(np.log2(np.maximum(mx, 1e-30))), 0.0)
    k = np.clip(k, -3, 3)
    sc = np.exp2(k).astype(np.float32)
    t = np.empty((N, CS), dtype=np.float32)
    t[:, 0:NHID] = xs * sc[:, None]
    t[:, NHID] = sc
    return t.astype(F8E3), (1.0 / sc)


def _streams(core, sched, table, yinv_n, s_n, d_n):
    """Build feats/meta/idx/dstl arrays for one core."""
    NT = sched["NT"]
    ssrc = core["slot_src"]
    feats = table[ssrc]                                   # [NSLOT, 65]
    feats = np.ascontiguousarray(
        feats.reshape(NT, P, CS).transpose(1, 0, 2))      # [P, NT, CS]
    pre = (s_n[ssrc] + d_n[core["slot_dst"]]).astype(np.float32)
    pre[core["pad"]] = -30000.0
    pre = pre.astype(BF16).reshape(NT, P).T               # [P, NT]
    yv = yinv_n[ssrc].astype(BF16).reshape(NT, P).T       # [P, NT]
    jj = core["slot_j"].reshape(NT, P).T                  # [P, NT]
    padm = core["pad"].reshape(NT, P).T
    meta = np.empty((P, 2 * NT), dtype=BF16)
    idx = np.full((P, sched["NIDX"]), -1, dtype=np.int16)
    dstl = np.zeros((P, sched["NDVE"]), dtype=BF16)
    for ch in sched["chunks"]:
        t0, TC = ch["t0"], ch["TC"]
        meta[:, 2 * t0:2 * t0 + TC] = pre[:, t0:t0 + TC]
        meta[:, 2 * t0 + TC:2 * t0 + 2 * TC] = yv[:, t0:t0 + TC]
        if ch["kind"] == "pool":
            for (tl, sT, ic0, icols) in ch["subs"]:
                a = t0 + tl
                v = (np.arange(sT)[None, :] * G + jj[:, a:a + sT]).astype(np.int16)
                v[padm[:, a:a + sT]] = -1
                idx[:, ch["idx_off"] + ic0:ch["idx_off"] + ic0 + sT] = v
        else:
            dstl[:, ch["dstl_off"]:ch["dstl_off"] + TC] = \
                jj[:, t0:t0 + TC].astype(BF16)
    return dict(feats=feats, meta=meta, idx=idx, dstl=dstl)


def _make_iota(sched):
    i = np.arange(G, dtype=np.float32)[None, :, None]
    return np.broadcast_to(i, (P, G, sched["TCMAXD"])).astype(BF16).copy()


def _run_lin(nc_lin, xT_list, W, a_src, a_dst):
    Wb = np.ascontiguousarray(W, dtype=np.float32).astype(BF16)
    WTb = np.ascontiguousarray(W.T, dtype=np.float32).astype(BF16)
    ap = np.stack([a_src, a_dst], axis=1).astype(np.float32).astype(BF16)
    in_maps = [{"xT": xT_list[c], "w": Wb, "wT": WTb, "apair": ap}
               for c in range(NCORES)]
    res = run_bass_kernel_spmd(nc_lin, in_maps, core_ids=list(range(NCORES)))
    xs = np.empty((N, NHID + 2), dtype=np.float32)
    for c in range(NCORES):
        xs[c * NSH:(c + 1) * NSH] = \
            res.results[c]["xs_sd"][:, :NSH].T.astype(np.float32)
    return xs[:, 0:NHID], xs[:, NHID], xs[:, NHID + 1]


def _run_agg(nc_agg, sched, cores, xs, s, d, fp8, iota):
    table, yinv_n = _quant_table(xs, fp8)
    in_maps = []
    for core in cores:
        st = _streams(core, sched, table, yinv_n, s, d)
        st["iota"] = iota
        in_maps.append(st)
    res = run_bass_kernel_spmd(nc_agg, in_maps, core_ids=list(range(NCORES)))
    full = np.zeros((N, NHID), dtype=np.float32)
    for c, core in enumerate(cores):
        o = res.results[c]["out"]                     # [P, NPS, 64] bf16
        rows = o.transpose(1, 0, 2).reshape(NB * G, NHID).astype(np.float32)
        valid = core["perm"] >= 0
        full[c * NSH + core["perm"][valid]] = rows[valid]
    return full


def kernel(x, W1, att_src1, att_dst1, W2, att_src2, att_dst2, edge_index):
    x = np.asarray(x, dtype=np.float32)
    W1 = np.asarray(W1, dtype=np.float32)
    W2 = np.asarray(W2, dtype=np.float32)
    a_s1 = np.asarray(att_src1, dtype=np.float32)
    a_d1 = np.asarray(att_dst1, dtype=np.float32)
    a_s2 = np.asarray(att_src2, dtype=np.float32)
    a_d2 = np.asarray(att_dst2, dtype=np.float32)

    sched, cores = _prep_graph(edge_index)
    iota = _make_iota(sched)
    NODES_PAD = NB * G

    ncA = _get(("lin", NFEAT), _build_lin, NFEAT)
    ncB2 = _get(("lin", NHID), _build_lin, NHID)
    ncB = _get(("agg", True), _build_agg, True, FP8_L1, sched)
    ncC = _get(("agg", False), _build_agg, False, FP8_L2, sched)

    # layer 1
    xb = x.astype(BF16)
    xT_list = []
    for c in range(NCORES):
        xt = np.zeros((NFEAT, NODES_PAD), dtype=BF16)
        xt[:, :NSH] = xb[c * NSH:(c + 1) * NSH].T
        xT_list.append(xt)
    xs1, s1, d1 = _run_lin(ncA, xT_list, W1, a_s1, a_d1)
    h = _run_agg(ncB, sched, cores, xs1, s1, d1, FP8_L1, iota)

    # layer 2
    hb = h.astype(BF16)
    hT_list = []
    for c in range(NCORES):
        ht = np.zeros((NHID, NODES_PAD), dtype=BF16)
        ht[:, :NSH] = hb[c * NSH:(c + 1) * NSH].T
        hT_list.append(ht)
    xs2, s2, d2 = _run_lin(ncB2, hT_list, W2, a_s2, a_d2)
    out = _run_agg(ncC, sched, cores, xs2, s2, d2, FP8_L2, iota)
    return out.astype(np.float32)
